# revision 17
# baseline (speedup 1.0000x reference)
"""GatedGCN message-passing kernel for 8 TRN2 NeuronCores (Bass/Tile).

Strategy: nodes partitioned contiguously across 8 cores (12500 each). Edges
assigned to the core owning dst. Per core, edges are grouped by src block
(4 blocks of 32768 rows so gather indices fit int16), dst-sorted within each
group, and padded so every 128-edge subtile belongs to a single 128-node dst
window. Per layer: hs = h[src] via 4-queue dma_gather; the hd@B term is
produced by an expansion matmul from the local node block (no gather); the
segment sums (num/den) are computed on-chip with one-hot segment matmuls
accumulated in PSUM (no scatter). h is refreshed across cores with AllGather.
"""
import math
import sys

import numpy as np

if "/opt/trn_rl_repo" not in sys.path:
    sys.path.insert(0, "/opt/trn_rl_repo")

P = 128
H = 64


# ----------------------------------------------------------------------------
# host-side planning
# ----------------------------------------------------------------------------

def _wrap16(a):
    """int array [EP] -> wrapped idx layout [128, EP/16] int16 (8x replicated)."""
    w = a.reshape(-1, 16).T.astype(np.int16)
    return np.ascontiguousarray(np.tile(w, (8, 1)))


def _colmaj(a):
    """[EP] -> [128, EP/128]: position i -> partition i%128, col i//128."""
    return np.ascontiguousarray(a.reshape(-1, 128).T)


def _stack2(a, chunk):
    """[EP, k] -> stacked [2k, EP/2] with INTERLEAVED subtile pairs: stacked
    col (c_chunk, q, e) row [half*k + f] = edge (c_chunk*chunk + (2q+half)*128 + e)."""
    ep, k = a.shape
    npair = chunk // 256
    t = a.reshape(ep // chunk, npair, 2, 128, k)     # [nc, q, half, e, k]
    t = np.transpose(t, (2, 4, 0, 1, 3))             # [half, k, nc, q, e]
    return np.ascontiguousarray(t.reshape(2 * k, ep // 2))


def _plan(src, dst, n_loc, n_win, blk, n_blocks, blk_rows, chunk, ncores):
    """Partition edges by dst owner; per core 4 src-block groups, dst-sorted,
    padded so each (group, window) occupies K_gw 128-edge subtiles (K_gw global
    across cores). Returns per-core edge placement + global structure."""
    core = dst // n_loc
    sub = P
    # per (core, block, window) counts
    K = np.zeros((n_blocks, n_win), np.int64)
    percore = []
    for c in range(ncores):
        ids = np.nonzero(core == c)[0].astype(np.int64)
        dl = dst[ids] - c * n_loc
        b = src[ids] // blk
        order = np.lexsort((dl, b))
        ids, dl, b = ids[order], dl[order], b[order]
        w = dl // sub
        percore.append((ids, dl, b, w))
        for bb in range(n_blocks):
            m = b == bb
            cnt = np.bincount(w[m], minlength=n_win)
            K[bb] = np.maximum(K[bb], (cnt + sub - 1) // sub)
    K = np.maximum(K, 0)
    # chunk-align each group by growing K of the last windows
    nsub_chunk = chunk // sub
    for bb in range(n_blocks):
        rem = (-int(K[bb].sum())) % nsub_chunk
        K[bb, n_win - 1] += rem  # pad subtiles appended to last window (all-pad)
    gs_sub = K.sum(axis=1)                      # subtiles per group
    ep = int(gs_sub.sum()) * sub                # padded edges per core
    # global subtile -> (block, window) map and window -> subtile range
    sub_block = np.concatenate([np.full(int(gs_sub[bb]), bb, np.int64)
                                for bb in range(n_blocks)])
    sub_win = np.concatenate([np.repeat(np.arange(n_win), K[bb])
                              for bb in range(n_blocks)])
    # first/last subtile flags per (block, window)
    n_sub = sub_block.shape[0]
    first = np.zeros(n_sub, bool)
    last = np.zeros(n_sub, bool)
    pos = 0
    for bb in range(n_blocks):
        for wv in range(n_win):
            k = int(K[bb, wv])
            if k:
                first[pos] = True
                last[pos + k - 1] = True
                pos += k
    # per-core arrays
    win_base = {}
    pos0 = np.zeros((n_blocks, n_win), np.int64)
    pos = 0
    for bb in range(n_blocks):
        for wv in range(n_win):
            pos0[bb, wv] = pos * sub
            pos += int(K[bb, wv])
    cores = []
    for c in range(ncores):
        ids, dl, b, w = percore[c]
        # rank within (b, w)
        key = b * n_win + w
        order = np.argsort(key, kind="stable")
        assert (order == np.arange(len(key))).all()  # already sorted
        # compute rank via cumcount
        rank = np.zeros(len(key), np.int64)
        if len(key):
            changes = np.r_[True, key[1:] != key[:-1]]
            seg_start = np.nonzero(changes)[0]
            rank = np.arange(len(key)) - np.repeat(seg_start, np.diff(np.r_[seg_start, len(key)]))
        gpos = pos0[b, w] + rank
        src_loc = np.zeros(ep, np.int64)
        drel = np.full(ep, -1.0, np.float32)
        eid = np.full(ep, -1, np.int64)
        src_loc[gpos] = src[ids] - b * blk
        drel[gpos] = (dl - w * sub).astype(np.float32)
        eid[gpos] = ids
        cores.append({"src_loc": src_loc, "drel": drel, "eid": eid})
    return {
        "K": K, "ep": ep, "gs_sub": gs_sub,
        "sub_block": sub_block, "sub_win": sub_win,
        "sub_first": first, "sub_last": last,
        "cores": cores,
    }


def _blockdiag(w):
    out = np.zeros((2 * w.shape[0], 2 * w.shape[1]), np.float32)
    out[:w.shape[0], :w.shape[1]] = w
    out[w.shape[0]:, w.shape[1]:] = w
    return out


# ----------------------------------------------------------------------------
# device program
# ----------------------------------------------------------------------------

def _build(cfg):
    import concourse.bacc as bacc
    import concourse.mybir as mybir
    import concourse.tile as tile
    fp = mybir.dt.float32
    i16 = mybir.dt.int16
    AF = mybir.ActivationFunctionType
    OP = mybir.AluOpType

    N, NLOC, NWIN = cfg["N"], cfg["NLOC"], cfg["NWIN"]
    NLOCP = NWIN * P
    L = cfg["L"]
    EP, EPS = cfg["EP"], cfg["EP"] // 2
    SP_, SPS = cfg["SP"], cfg["SP"] // 2
    CH = cfg["CHUNK"]
    NSUB = CH // P
    COLS = CH // 2
    NT = COLS // 512 if COLS >= 512 else 1
    TW = min(512, COLS)           # psum tile width
    blk_rows = cfg["blk_rows"]
    e_sub_block, e_sub_win = cfg["e_sub_block"], cfg["e_sub_win"]
    e_first, e_last = cfg["e_first"], cfg["e_last"]
    s_sub_block, s_sub_win = cfg["s_sub_block"], cfg["s_sub_win"]
    blk0 = np.cumsum([0] + list(blk_rows))

    nc = bacc.Bacc("TRN2", target_bir_lowering=False, debug=False,
                   num_devices=8, num_swdge_queues=4)

    def inp(name, shape, dt=fp):
        return nc.dram_tensor(name, list(shape), dt, kind="ExternalInput").ap()

    xT = inp("xT", [2, NLOCP])
    eraw = inp("eraw", [4, EPS])
    gsrc = inp("gsrc", [P, EP // 16], i16)
    gdrel = inp("gdrel", [P, EP // P])
    ssrc = inp("ssrc", [P, SP_ // 16], i16)
    sdrel = inp("sdrel", [P, SP_ // P])
    w_Abd = inp("w_Abd", [L, P, P])
    w_Cbd = inp("w_Cbd", [L, P, P])       # layer 0 slot: (We@C0) padded into [128,128] top-left [4,128]? no: separate
    w_C0p = inp("w_C0p", [4, P])
    w_B = inp("w_B", [L, P, H])           # replicated rows (B; B)
    w_U = inp("w_U", [L, P, H])
    w_Vbd = inp("w_Vbd", [L, P, P])
    w_cb = inp("w_cb", [L, P, 1])
    w_Wn = inp("w_Wn", [2, H])
    w_bn = inp("w_bn", [P, H])
    w_Wp1 = inp("w_Wp1", [P, 2])
    w_Wp2 = inp("w_Wp2", [P, 1])          # replicated (Wp2; Wp2)
    w_bp = inp("w_bp", [2, 1])
    w_id = inp("w_id", [P, P])
    w_iota = inp("w_iota", [P, P])

    scores = nc.dram_tensor("scores", [2, SPS], fp, kind="ExternalOutput").ap()
    DBG = bool(cfg.get("debug"))
    if DBG:
        dbg_agg = nc.dram_tensor("dbg_agg", [P, NWIN * 2 * H], fp, kind="ExternalOutput").ap()
        dbg_sem = nc.dram_tensor("dbg_sem", [P, 2 * NSUB * P], fp, kind="ExternalOutput").ap()
        dbg_rhs = nc.dram_tensor("dbg_rhs", [P, 2 * NSUB * P], fp, kind="ExternalOutput").ap()
        dbg_hsT = nc.dram_tensor("dbg_hsT", [P, COLS], fp, kind="ExternalOutput").ap()
        dbg_sig = nc.dram_tensor("dbg_sig", [P, COLS], fp, kind="ExternalOutput").ap()
        dbg_val = nc.dram_tensor("dbg_val", [P, COLS], fp, kind="ExternalOutput").ap()

    h_table = nc.dram_tensor("h_table", [N, H], fp).ap()
    h_loc_dram = nc.dram_tensor("h_loc_dram", [NLOCP, H], fp).ap()
    e_a = nc.dram_tensor("e_a", [P, EPS], fp).ap()
    e_b = nc.dram_tensor("e_b", [P, EPS], fp).ap()
    e_bufs = [None, e_a, e_b]

    with tile.TileContext(nc) as tc:
        with (
            tc.tile_pool(name="wp", bufs=1) as wp,
            tc.tile_pool(name="hp", bufs=1) as hp,
            tc.tile_pool(name="cp", bufs=2) as cp,
            tc.tile_pool(name="pp", bufs=2, space="PSUM") as pp,
            tc.tile_pool(name="pe", bufs=1, space="PSUM") as pe,
            tc.tile_pool(name="up", bufs=2) as up,
        ):
            # ---- constants to SBUF
            def const(name, ap_, shape, dt=fp):
                t = wp.tile(list(shape), dt, tag=name)
                nc.sync.dma_start(t[:], ap_)
                return t
            xT_sb = const("xT", xT[:], [2, NLOCP])
            Abd = [const(f"A{i}", w_Abd[i], [P, P]) for i in range(L)]
            Cbd = [const(f"C{i}", w_Cbd[i], [P, P]) for i in range(L)]
            C0p = const("C0p", w_C0p[:], [4, P])
            Bw = [const(f"B{i}", w_B[i], [P, H]) for i in range(L)]
            Uw = [const(f"U{i}", w_U[i], [P, H]) for i in range(L)]
            Vbd = [const(f"V{i}", w_Vbd[i], [P, P]) for i in range(L)]
            cb = [const(f"cb{i}", w_cb[i], [P, 1]) for i in range(L)]
            Wn = const("Wn", w_Wn[:], [2, H])
            bn = const("bn", w_bn[:], [P, H])
            Wp1 = const("Wp1", w_Wp1[:], [P, 2])
            Wp2 = const("Wp2", w_Wp2[:], [P, 1])
            bp = const("bp", w_bp[:], [2, 1])
            ident = const("ident", w_id[:], [P, P])
            iota = const("iota", w_iota[:], [P, P])

            h_loc = hp.tile([P, NWIN, H], fp, tag="h_loc")
            hT_st = hp.tile([P, (NWIN + 1) // 2 * P], fp, tag="hT_st")
            hB_sb = hp.tile([P, NWIN, H], fp, tag="hB_sb")
            agg = hp.tile([P, NWIN, 2 * H], fp, tag="agg")
            HW2 = (NWIN + 1) // 2

            # ---- encoder: h0 = x@Wn + bn (node-major windows)
            for w in range(NWIN):
                ps = pe.tile([P, H], fp, tag="sm_ps", space="PSUM")
                nc.tensor.matmul(ps[:], xT_sb[:, w * P:(w + 1) * P], Wn[0:2, :],
                                 start=True, stop=True)
                nc.vector.tensor_add(h_loc[:, w, :], ps[:], bn[:])
            nc.sync.dma_start(
                h_loc_dram[:].rearrange("(w p) f -> p w f", p=P), h_loc[:])
            nc.gpsimd.collective_compute(
                "AllGather", mybir.AluOpType.bypass,
                replica_groups=[list(range(8))],
                ins=[h_loc_dram[0:NLOC, :].opt()], outs=[h_table[:].opt()])

            # ---- layers
            for li in range(L):
                # h_locT stacked + hB per window
                for w in range(NWIN):
                    half, wc = (0, w) if w < HW2 else (1, w - HW2)
                    tp = pe.tile([H, P], fp, tag="sm_ps", space="PSUM")
                    nc.tensor.transpose(tp[:], h_loc[:, w, :], ident[:])
                    nc.vector.tensor_copy(
                        hT_st[half * H:(half + 1) * H, wc * P:(wc + 1) * P], tp[:])
                for w in range(NWIN):
                    half, wc = (0, w) if w < HW2 else (1, w - HW2)
                    hb_ps = pe.tile([P, H], fp, tag="sm_ps", space="PSUM")
                    nc.tensor.matmul(
                        hb_ps[:],
                        hT_st[half * H:(half + 1) * H, wc * P:(wc + 1) * P],
                        Bw[li][half * H:(half + 1) * H, :], start=True, stop=True)
                    nc.scalar.activation(hB_sb[:, w, :], hb_ps[:], AF.Copy)
                nc.vector.memset(agg[:], 0.0)

                # edge chunks
                nchunks = EP // CH
                agg_ps_open = {}
                for k in range(nchunks):
                    sub0 = k * NSUB
                    # loads
                    idx_t = cp.tile([P, CH // 16], i16, tag="gidx")
                    nc.sync.dma_start(idx_t[:], gsrc[:, k * (CH // 16):(k + 1) * (CH // 16)])
                    drel_t = cp.tile([P, NSUB], fp, tag="drel")
                    nc.sync.dma_start(drel_t[:], gdrel[:, k * NSUB:(k + 1) * NSUB])
                    bb = int(e_sub_block[sub0])
                    g_t = cp.tile([P, NSUB, H], fp, tag="hsg")
                    nc.gpsimd.dma_gather(
                        g_t[:], h_table[int(blk0[bb]):int(blk0[bb + 1]), :], idx_t[:],
                        CH, CH, H, single_packet=False, queue_num=k % 4)
                    if li == 0:
                        er_t = cp.tile([4, COLS], fp, tag="eraw")
                        nc.sync.dma_start(er_t[:], eraw[:, k * COLS:(k + 1) * COLS])
                    else:
                        eT_t = cp.tile([P, COLS], fp, tag="eT")
                        nc.sync.dma_start(eT_t[:], e_bufs[li][:, k * COLS:(k + 1) * COLS])

                    # T-in: hs pairs -> stacked hsT
                    hsT = cp.tile([P, COLS], fp, tag="hsT")
                    for t in range(NT):
                        tp = pp.tile([P, TW], fp, tag="t_ps", space="PSUM")
                        for q in range(TW // P):
                            j = t * (TW // P) + q
                            nc.tensor.transpose(
                                tp[:, q * P:(q + 1) * P],
                                g_t[:].rearrange("p a h -> p (a h)")
                                [:, j * P:(j + 1) * P], ident[:])
                        nc.scalar.activation(hsT[:, t * TW:(t + 1) * TW], tp[:], AF.Copy)

                    # S_em per subtile + S_T via transpose
                    S_em = cp.tile([P, NSUB, P], fp, tag="S_em")
                    for j in range(NSUB):
                        nc.vector.tensor_tensor(
                            out=S_em[:, j, :],
                            in0=drel_t[:, j:j + 1].to_broadcast([P, P]),
                            in1=iota[:], op=OP.is_equal)
                    S_T = cp.tile([P, NSUB, P], fp, tag="S_T")
                    for t0 in range(0, NSUB, 4):
                        nq = min(4, NSUB - t0)
                        stp = pp.tile([P, 4 * P], fp, tag="t_ps", space="PSUM")
                        for q in range(nq):
                            nc.tensor.transpose(stp[:, q * P:(q + 1) * P],
                                                S_em[:, t0 + q, :], ident[:])
                        nc.scalar.activation(
                            S_T[:].rearrange("p s e -> p (s e)")
                            [:, t0 * P:(t0 + nq) * P], stp[:, 0:nq * P], AF.Copy)

                    # e_hat psums + sigma + e'
                    sig_t = cp.tile([P, COLS], fp, tag="sig")
                    val_t = cp.tile([P, COLS], fp, tag="val")
                    if li < L - 1:
                        ep_t = cp.tile([P, COLS], fp, tag="epT")
                    for t in range(NT):
                        cs = slice(t * TW, (t + 1) * TW)
                        eh = pp.tile([P, TW], fp, tag="eh_ps", space="PSUM")
                        nsub_t = TW // P
                        nc.tensor.matmul(eh[:], Abd[li][:], hsT[:, cs], start=True,
                                         stop=False, skip_group_check=True)
                        for q in range(nsub_t):
                            for half in range(2):
                                j = 2 * (t * nsub_t + q) + half
                                w = int(e_sub_win[sub0 + j])
                                nc.tensor.matmul(
                                    eh[half * H:(half + 1) * H, q * P:(q + 1) * P],
                                    hB_sb[:, w, :], S_T[:, j, :],
                                    start=False, stop=False, skip_group_check=True)
                        if li == 0:
                            nc.tensor.matmul(eh[:], C0p[:], er_t[:, cs], start=False,
                                             stop=True, skip_group_check=True)
                        else:
                            nc.tensor.matmul(eh[:], Cbd[li][:], eT_t[:, cs], start=False,
                                             stop=True, skip_group_check=True)
                        nc.scalar.activation(sig_t[:, cs], eh[:], AF.Sigmoid, bias=cb[li][:])
                        if li < L - 1:
                            nc.scalar.activation(ep_t[:, cs], eh[:], AF.Relu, bias=cb[li][:])
                        hv = pp.tile([P, TW], fp, tag="hv_ps", space="PSUM")
                        nc.tensor.matmul(hv[:], Vbd[li][:], hsT[:, cs], start=True, stop=True)
                        nc.vector.tensor_mul(val_t[:, cs], sig_t[:, cs], hv[:])
                    if li < L - 1:
                        nc.sync.dma_start(
                            e_bufs[li + 1][:, k * COLS:(k + 1) * COLS], ep_t[:])

                    # T-back val/sig -> edge-major rhs
                    rhs = cp.tile([P, NSUB, P], fp, tag="rhs")
                    npairs = NSUB // 2
                    for src_t, off in ((val_t, 0), (sig_t, H)):
                        for t0 in range(0, npairs, 4):
                            npq = min(4, npairs - t0)
                            vb = pp.tile([P, 4 * P], fp, tag="t_ps", space="PSUM")
                            for q in range(npq):
                                j = t0 + q
                                nc.tensor.transpose(vb[:, q * P:(q + 1) * P],
                                                    src_t[:, j * P:(j + 1) * P], ident[:])
                            vbv = vb[:].rearrange("p (ah f) -> p ah f", f=H)
                            nc.vector.tensor_copy(
                                rhs[:, 2 * t0:2 * t0 + 2 * npq, off:off + H],
                                vbv[:, 0:2 * npq, :])

                    # aggregation: per-chunk runs of same-window subtiles
                    j = 0
                    while j < NSUB:
                        w = int(e_sub_win[sub0 + j])
                        j2 = j
                        while j2 + 1 < NSUB and int(e_sub_win[sub0 + j2 + 1]) == w:
                            j2 += 1
                        ap_ = pe.tile([P, 2 * H], fp, tag="agg_ps",
                                      name=f"aggps_{li}_{k}_{j}", space="PSUM")
                        for jj in range(j, j2 + 1):
                            nc.tensor.matmul(ap_[:], S_em[:, jj, :], rhs[:, jj, :],
                                             start=(jj == j), stop=(jj == j2))
                        nc.vector.tensor_add(agg[:, w, :], agg[:, w, :], ap_[:])
                        j = j2 + 1
                    if DBG and li == 0 and k == 0:
                        nc.sync.dma_start(dbg_hsT[:], hsT[:])
                        nc.sync.dma_start(dbg_sig[:], sig_t[:])
                        nc.sync.dma_start(dbg_val[:], val_t[:])
                    if DBG and li == 0 and k < 2:
                        nc.sync.dma_start(
                            dbg_sem[:, k * NSUB * P:(k + 1) * NSUB * P],
                            S_em[:].rearrange("p a b -> p (a b)"))
                        nc.sync.dma_start(
                            dbg_rhs[:, k * NSUB * P:(k + 1) * NSUB * P],
                            rhs[:].rearrange("p a b -> p (a b)"))

                if DBG and li == 0:
                    nc.sync.dma_start(dbg_agg[:],
                                      agg[:].rearrange("p a b -> p (a b)"))
                # update h
                for w in range(NWIN):
                    half, wc = (0, w) if w < HW2 else (1, w - HW2)
                    dene = up.tile([P, H], fp, tag="dene")
                    nc.scalar.activation(dene[:], agg[:, w, H:2 * H], AF.Copy, bias=1e-6)
                    rec = up.tile([P, H], fp, tag="rec")
                    nc.vector.reciprocal(rec[:], dene[:])
                    qt = up.tile([P, H], fp, tag="qt")
                    nc.vector.tensor_mul(qt[:], agg[:, w, 0:H], rec[:])
                    hu = pe.tile([P, H], fp, tag="sm_ps", space="PSUM")
                    nc.tensor.matmul(
                        hu[:], hT_st[half * H:(half + 1) * H, wc * P:(wc + 1) * P],
                        Uw[li][half * H:(half + 1) * H, :], start=True, stop=True)
                    ht = up.tile([P, H], fp, tag="ht")
                    nc.vector.tensor_add(ht[:], hu[:], qt[:])
                    nc.scalar.activation(h_loc[:, w, :], ht[:], AF.Relu)
                nc.sync.dma_start(
                    h_loc_dram[:].rearrange("(w p) f -> p w f", p=P), h_loc[:])
                nc.gpsimd.collective_compute(
                    "AllGather", mybir.AluOpType.bypass,
                    replica_groups=[list(range(8))],
                    ins=[h_loc_dram[0:NLOC, :].opt()], outs=[h_table[:].opt()])

            # ---- scorer
            # final h_locT + hWp2 per window
            for w in range(NWIN):
                half, wc = (0, w) if w < HW2 else (1, w - HW2)
                tp = pe.tile([H, P], fp, tag="sm_ps", space="PSUM")
                nc.tensor.transpose(tp[:], h_loc[:, w, :], ident[:])
                nc.vector.tensor_copy(
                    hT_st[half * H:(half + 1) * H, wc * P:(wc + 1) * P], tp[:])
            hWp2 = hp.tile([P, NWIN, 1], fp, tag="hWp2")
            for w in range(NWIN):
                half, wc = (0, w) if w < HW2 else (1, w - HW2)
                ps = pe.tile([P, 1], fp, tag="sm_ps", space="PSUM")
                nc.tensor.matmul(ps[:], hT_st[half * H:(half + 1) * H, wc * P:(wc + 1) * P],
                                 Wp2[half * H:(half + 1) * H, :], start=True, stop=True)
                nc.vector.tensor_copy(hWp2[:, w, :], ps[:])

            nschunks = SP_ // CH
            for k in range(nschunks):
                sub0 = k * NSUB
                idx_t = cp.tile([P, CH // 16], i16, tag="sidx")
                nc.sync.dma_start(idx_t[:], ssrc[:, k * (CH // 16):(k + 1) * (CH // 16)])
                drel_t = cp.tile([P, NSUB], fp, tag="sdrel")
                nc.sync.dma_start(drel_t[:], sdrel[:, k * NSUB:(k + 1) * NSUB])
                bb = int(s_sub_block[sub0])
                g_t = cp.tile([P, NSUB, H], fp, tag="ssg")
                nc.gpsimd.dma_gather(
                    g_t[:], h_table[int(blk0[bb]):int(blk0[bb + 1]), :], idx_t[:],
                    CH, CH, H, single_packet=False, queue_num=k % 4)
                hsT = cp.tile([P, COLS], fp, tag="shsT")
                for t in range(NT):
                    tp = pp.tile([P, TW], fp, tag="t_ps", space="PSUM")
                    for q in range(TW // P):
                        j = t * (TW // P) + q
                        nc.tensor.transpose(
                            tp[:, q * P:(q + 1) * P],
                            g_t[:].rearrange("p a h -> p (a h)")
                            [:, j * P:(j + 1) * P], ident[:])
                    nc.scalar.activation(hsT[:, t * TW:(t + 1) * TW], tp[:], AF.Copy)
                S_T = cp.tile([P, NSUB, P], fp, tag="sS_T")
                for j in range(NSUB):
                    se = cp.tile([P, P], fp, tag="sS_em")
                    nc.vector.tensor_tensor(
                        out=se[:], in0=drel_t[:, j:j + 1].to_broadcast([P, P]),
                        in1=iota[:], op=OP.is_equal)
                    stp = pp.tile([P, P], fp, tag="t_ps", space="PSUM")
                    nc.tensor.transpose(stp[:], se[:], ident[:])
                    nc.scalar.activation(S_T[:, j, :], stp[:], AF.Copy)
                sc_ev = cp.tile([1, COLS], fp, tag="sc_ev")
                sc_od = cp.tile([1, COLS], fp, tag="sc_od")
                for t in range(NT):
                    cs = slice(t * TW, (t + 1) * TW)
                    sh_e = pp.tile([1, TW], fp, tag="eh_ps", name=f"she{k}_{t}", space="PSUM")
                    nc.tensor.matmul(sh_e[:], Wp1[:, 0:1], hsT[:, cs], start=True, stop=True)
                    sh_o = pp.tile([1, TW], fp, tag="hv_ps", name=f"sho{k}_{t}", space="PSUM")
                    nc.tensor.matmul(sh_o[:], Wp1[:, 1:2], hsT[:, cs], start=True, stop=True)
                    sce = pe.tile([1, TW], fp, tag="sm_ps", name=f"sce{k}_{t}", space="PSUM")
                    sco = pe.tile([1, TW], fp, tag="agg_ps", name=f"sco{k}_{t}", space="PSUM")
                    for q in range(TW // P):
                        for half in range(2):
                            j = 2 * (t * (TW // P) + q) + half
                            w = int(s_sub_win[sub0 + j])
                            tgt = sce if half == 0 else sco
                            nc.tensor.matmul(tgt[:, q * P:(q + 1) * P], hWp2[:, w, :],
                                             S_T[:, j, :], start=True, stop=True)
                    she_sb = cp.tile([1, TW], fp, tag="she_sb")
                    nc.scalar.activation(she_sb[:], sh_e[:], AF.Identity, bias=bp[0:1, :])
                    sho_sb = cp.tile([1, TW], fp, tag="sho_sb")
                    nc.scalar.activation(sho_sb[:], sh_o[:], AF.Identity, bias=bp[0:1, :])
                    nc.vector.tensor_add(sc_ev[0:1, cs], she_sb[:], sce[:])
                    nc.vector.tensor_add(sc_od[0:1, cs], sho_sb[:], sco[:])
                nc.sync.dma_start(scores[0:1, k * COLS:(k + 1) * COLS], sc_ev[:])
                nc.sync.dma_start(scores[1:2, k * COLS:(k + 1) * COLS], sc_od[:])

    nc.compile()
    return nc


# ----------------------------------------------------------------------------
# entry point
# ----------------------------------------------------------------------------

_CACHE = {}


def _get_dims(x, e, src, src_s):
    return dict(N=x.shape[0], E=src.shape[0], ES=src_s.shape[0])


def _prepare(x, e, Wn, bn, We, be, A, B, C, cb, U, V, Wp, bp, src, dst,
             src_s, dst_s, chunk):
    x = np.asarray(e) if False else np.asarray(x); e = np.asarray(e)
    src = np.asarray(src).astype(np.int64)
    dst = np.asarray(dst).astype(np.int64)
    src_s = np.asarray(src_s).astype(np.int64)
    dst_s = np.asarray(dst_s).astype(np.int64)
    A, B, C, cb, U, V = (np.asarray(a, np.float32) for a in (A, B, C, cb, U, V))
    Wn, bn, We, be, Wp, bp = (np.asarray(a, np.float32) for a in (Wn, bn, We, be, Wp, bp))

    N, E, ES = x.shape[0], src.shape[0], src_s.shape[0]
    L = A.shape[0]
    NCORE = 8
    NLOC = N // NCORE
    NWIN = (NLOC + P - 1) // P
    NLOCP = NWIN * P
    BLK = 32768
    NBLK = (N + BLK - 1) // BLK
    blk_rows = [min(BLK, N - b * BLK) for b in range(NBLK)]

    eplan = _plan(src, dst, NLOC, NWIN, BLK, NBLK, blk_rows, chunk, NCORE)
    splan = _plan(src_s, dst_s, NLOC, NWIN, BLK, NBLK, blk_rows, chunk, NCORE)
    EP, SPP = eplan["ep"], splan["ep"]

    cfg = dict(N=N, NLOC=NLOC, NWIN=NWIN, L=L, EP=EP, SP=SPP, CHUNK=chunk,
               blk_rows=blk_rows,
               e_sub_block=eplan["sub_block"], e_sub_win=eplan["sub_win"],
               e_first=eplan["sub_first"], e_last=eplan["sub_last"],
               s_sub_block=splan["sub_block"], s_sub_win=splan["sub_win"])

    key = (N, E, ES, L, EP, SPP, chunk,
           eplan["sub_win"].tobytes(), splan["sub_win"].tobytes(),
           eplan["sub_block"].tobytes(), splan["sub_block"].tobytes())
    if key not in _CACHE:
        _CACHE[key] = _build(cfg)
    nc = _CACHE[key]

    # weights (shared across cores)
    C0p = (We @ C[0]).astype(np.float32)           # [2, 64]
    cb0p = (be @ C[0] + cb[0]).astype(np.float32)
    C0p_bd = np.zeros((4, P), np.float32)
    C0p_bd[0:2, 0:H] = C0p
    C0p_bd[2:4, H:2 * H] = C0p
    cbs = []
    for i in range(L):
        c_i = cb0p if i == 0 else cb[i]
        cbs.append(np.concatenate([c_i, c_i]).reshape(P, 1))
    wmaps = {
        "w_Abd": np.stack([_blockdiag(A[i]) for i in range(L)]),
        "w_Cbd": np.stack([_blockdiag(C[i]) for i in range(L)]),
        "w_C0p": C0p_bd,
        "w_B": np.stack([np.concatenate([B[i], B[i]], 0) for i in range(L)]),
        "w_U": np.stack([np.concatenate([U[i], U[i]], 0) for i in range(L)]),
        "w_Vbd": np.stack([_blockdiag(V[i]) for i in range(L)]),
        "w_cb": np.stack(cbs),
        "w_Wn": Wn,
        "w_bn": np.tile(bn.reshape(1, H), (P, 1)),
        "w_Wp1": np.concatenate(
            [np.concatenate([Wp[0:H], np.zeros((H, 1), np.float32)], 1),
             np.concatenate([np.zeros((H, 1), np.float32), Wp[0:H]], 1)], 0),
        "w_Wp2": np.concatenate([Wp[H:2 * H], Wp[H:2 * H]], 0),
        "w_bp": np.full((2, 1), float(bp[0]), np.float32),
        "w_id": np.eye(P, dtype=np.float32),
        "w_iota": np.tile(np.arange(P, dtype=np.float32), (P, 1)),
    }

    in_maps = []
    for c in range(NCORE):
        ec, sc = eplan["cores"][c], splan["cores"][c]
        xr = np.zeros((NLOCP, 2), np.float32)
        xr[0:NLOC] = x[c * NLOC:(c + 1) * NLOC]
        er = np.zeros((EP, 2), np.float32)
        m = ec["eid"] >= 0
        er[m] = e[ec["eid"][m]]
        im = dict(wmaps)
        im["xT"] = np.ascontiguousarray(xr.T)
        im["eraw"] = _stack2(er, chunk)
        im["gsrc"] = _wrap16(ec["src_loc"])
        im["gdrel"] = _colmaj(ec["drel"])
        im["ssrc"] = _wrap16(sc["src_loc"])
        im["sdrel"] = _colmaj(sc["drel"])
        in_maps.append(im)

    return nc, in_maps, splan, ES


def _unscore(score_arrays, splan, ES, chunk):
    out = np.zeros((ES, 1), np.float32)
    half = chunk // 2
    for c in range(len(score_arrays)):
        sc_arr = score_arrays[c]                   # [2, SPS]
        eid = splan["cores"][c]["eid"]
        pos = np.nonzero(eid >= 0)[0]
        ch_, u = pos // chunk, pos % chunk
        sub_j = u // 128
        rows = sub_j % 2
        cols = ch_ * half + (sub_j // 2) * 128 + (u % 128)
        out[eid[pos], 0] = sc_arr[rows, cols]
    return out


def kernel(x, e, Wn, bn, We, be, A, B, C, cb, U, V, Wp, bp, src, dst,
           src_s, dst_s, chunk=2048, _trace=False):
    from concourse.bass_utils import run_bass_kernel_spmd
    nc, in_maps, splan, ES = _prepare(
        x, e, Wn, bn, We, be, A, B, C, cb, U, V, Wp, bp, src, dst,
        src_s, dst_s, chunk)
    res = run_bass_kernel_spmd(nc, in_maps, list(range(8)), trace=_trace)
    kernel._last_result = res
    return _unscore([res.results[c]["scores"] for c in range(8)], splan, ES, chunk)
